# revision 90
# baseline (speedup 1.0000x reference)
"""Fused LayerNorm + MHA + out-proj for Trainium2, SPMD across 8 NeuronCores.

Problem: x[2,2048,1024] -> LN -> qkv (w_qkv[1024,3072]) -> 16-head attention
(dim_head 64) -> out proj (w_out[1024,1024] + b_out).

Sharding: core c handles batch c//4 and head-quad c%4 (heads 4*(c%4)..+4).
Each core: LN + transpose of its batch (replicated within the batch group),
qkv for its 4 heads, full attention for its 4 (b,h) pairs, then PER-HEAD
8-way AllToAlls redistribute head outputs: core c owns output rows
[256c, 256c+256) of EACH batch. Each core then computes the final projection
for its 512 rows locally.

Key structure (v2, ~234us cost-model vs 280us for the v1 head-pair layout):
- phase 1 split into stats-first stage A (SWDGE cast-DMA + bn_stats +
  in-place centering for all 16 row tiles; recip/center deferred one tile to
  hide the ACT-sqrt round trip, except tile 3 which gates PE start) and
  stage B (PE transpose + ACT affine + qkv), so the in-order DVE queue never
  holds the PSUM ring hostage;
- dep-free bulk DMAs (w_out, v_aug memsets) are WAW-gated behind phase-1
  tiles: the list scheduler otherwise hoists them to t=0 where they starve
  the x input stream on the shared DMA engines;
- the qkv v-path (psum -> vt -> transpose -> v_aug) is deferred one (ch,X)
  step and drained on ACT so PE never waits the drain round trip;
- attention per single head (not head pair) with per-key-tile score tiles
  (1 PSUM bank, ring 6), softmax exp split ACT 10 : DVE 6 per chunk (ACT hw
  exp / DVE Schraudolph int16 fast exp; Pool cannot read PSUM on HW), exp
  output ring 10, and the denominator multiply deferred one chunk so DVE
  never stalls on the Pool partition-broadcast -- coarser exp tiling
  otherwise paces the whole attention phase;
- per-head AllToAll (4 x 256KB collectives instead of 2 x 512KB): each
  collective issues ~29us after the previous, and the last one overlaps the
  out-proj pass A;
- out-proj: pass A (head pair 0 inner tiles + bias) right after attention
  under the in-flight last A2A; pass B (pair 1) after it lands, with the
  h==3 outT assembly split into 4 staggered DMAs AND the pass-B matmuls
  split into 32-col slivers: the cold-p-state pricing after the collective
  wait covers a fixed number of queued instructions, so smaller instructions
  spend proportionally less time at the throttled clock. The output ships
  bf16 (halves the final drain/DMA bytes; the host unshard casts to f32).
"""
import sys
sys.path.insert(0, '/opt/trn_rl_repo')
import numpy as np

import concourse.bass as bass
import concourse.tile as tile
import concourse.mybir as mybir
from concourse import bacc
from concourse.bass_utils import run_bass_kernel_spmd
from concourse.masks import make_identity

F32 = mybir.dt.float32
F32R = mybir.dt.float32r
BF16 = mybir.dt.bfloat16
AF = mybir.ActivationFunctionType
ALU = mybir.AluOpType

N_CORES = 8
B, N, DIM = 2, 2048, 1024
HEADS, DHEAD = 16, 64
H_LOC = 4                    # heads per core
ROWS = N                     # rows per core (one batch)
DT = DIM // 128              # 8 dim tiles
RCHUNK = 512
N_CH = ROWS // RCHUNK        # 4 row chunks
NKT = N // 128               # 16 key tiles
NQC = N // 512               # 4 query chunks
SCALE = DHEAD ** -0.5
EPS = 1e-5
# bf16-space Schraudolph fast exp: bitcast_bf16(int16(s*A + B)) ~ exp(SCALE*s)
A_SCH = SCALE * 128.0 / float(np.log(2.0))
B_SCH = 127.0 * 128.0 - 7.5
I16 = mybir.dt.int16

_CACHED_NC = None


def build():
    nc = bacc.Bacc("TRN2", target_bir_lowering=False, debug=False,
                   num_devices=N_CORES)
    x_ext = nc.dram_tensor("x", [ROWS, DIM], F32, kind="ExternalInput")
    gamma_ext = nc.dram_tensor("gamma", [DIM], F32, kind="ExternalInput")
    beta_ext = nc.dram_tensor("beta", [DIM], F32, kind="ExternalInput")
    wqkv_ext = nc.dram_tensor("wqkv", [DIM, 3 * H_LOC * DHEAD], F32,
                              kind="ExternalInput")
    wout_ext = nc.dram_tensor("wout", [DIM, DIM], F32, kind="ExternalInput")
    bout_ext = nc.dram_tensor("bout", [DIM], F32, kind="ExternalInput")
    out_ext = nc.dram_tensor("out", [RCHUNK, DIM], BF16,
                             kind="ExternalOutput")

    with tile.TileContext(nc) as tc:
        with tc.tile_pool(name="singles", bufs=1) as singles, \
             tc.tile_pool(name="xin", bufs=16) as xin, \
             tc.tile_pool(name="lnxt", bufs=3) as lnxtp, \
             tc.tile_pool(name="stats", bufs=8) as statsp, \
             tc.tile_pool(name="vt", bufs=3) as vtp, \
             tc.tile_pool(name="exps", bufs=10) as expsp, \
             tc.tile_pool(name="div", bufs=2) as divp, \
             tc.tile_pool(name="osb", bufs=2) as osbp, \
             tc.tile_pool(name="dram", bufs=1, space="DRAM") as dram:

            # ---------------- constants / weights ----------------
            ident_f = singles.tile([128, 128], F32)
            make_identity(nc, ident_f)
            ident = singles.tile([128, 128], BF16)
            nc.vector.tensor_copy(ident, ident_f)

            # gamma/beta as [128, DT] (partition p, dim tile dt -> dim dt*128+p)
            gamma_sb = singles.tile([128, DT], F32)
            beta_sb = singles.tile([128, DT], F32)

            nc.sync.dma_start(out=gamma_sb,
                              in_=gamma_ext.ap().rearrange("(dt p) -> p dt",
                                                           p=128))
            nc.sync.dma_start(out=beta_sb,
                              in_=beta_ext.ap().rearrange("(dt p) -> p dt",
                                                          p=128))
            eps_sb = singles.tile([128, 1], F32)
            nc.vector.memset(eps_sb, EPS)

            # w_qkv slice as bf16 lhsT tiles via SWDGE cast DMA, split in two
            # so the x input stream (which gates LayerNorm) interleaves
            w_sb = singles.tile([128, DT, 3 * H_LOC * DHEAD], BF16)
            # w_out bf16 via SWDGE cast DMA (walrus rejects mixed bf16/f32r
            # matmul operands); the load is WAW-gated to run after the x
            # input stream, in Pool's idle window before attention
            wo_sb = singles.tile([128, DT, DIM], BF16)

            # b_out f32 load + bf16 cast; ones lhsT [1, 128] for broadcast-add
            bo_f32 = singles.tile([1, DIM], F32)
            bo_sb = singles.tile([1, DIM], BF16)
            ones_bf = singles.tile([1, 128], BF16)
            nc.vector.memset(ones_bf, 1.0)

            # persistent activations
            qT = [singles.tile([128, ROWS], BF16, name=f"qT{i}") for i in range(2)]
            kT = [singles.tile([128, ROWS], BF16, name=f"kT{i}") for i in range(2)]
            # v_aug[:, h, t, 0]=1.0 (softmax denom), cols 1:64 zero pad,
            # [:, h, t, 64:128]=v -> attn@v psum row 0 = denom, rows 64:128 =
            # head outputs. (memsets emitted after stage A: the Pool queue
            # must start x desc-gen immediately and the DVE queue bn_stats)
            v_aug = singles.tile([128, H_LOC, NKT, 128], BF16)
            # head outputs (transposed): pair X holds heads 2X, 2X+1 in free dim
            oh = [singles.tile([128, 2, ROWS], BF16, name=f"oh{i}") for i in range(2)]

            # ---------------- phase 1 stage A: LN stats + centering --------
            ps1_cm = tc.tile_pool(name="ps1", bufs=6, space="PSUM")
            ps_ov_cm = tc.tile_pool(name="psov", bufs=2, space="PSUM")
            ps1 = ps1_cm.__enter__(); ps_ov = ps_ov_cm.__enter__()

            xts = []
            lnxts = []
            pend_center = None

            def center_flush():
                # recip+centering deferred one tile (the DVE queue runs the
                # next tile's bn_stats while ACT's sqrt is in flight), then
                # the LN transpose via the DMA xbar (16-bit) straight into
                # the chunk's lnxt tile: takes the 4 transposes per row tile
                # off the PE critical path entirely.
                px, pmv, prstd, pi = pend_center
                ti = pi + 1
                nc.vector.reciprocal(out=prstd, in_=prstd)
                with nc.allow_low_precision(reason="bf16 activations"):
                    nc.vector.tensor_scalar(out=px, in0=px,
                                            scalar1=pmv[:, 0:1],
                                            scalar2=prstd,
                                            op0=ALU.subtract,
                                            op1=ALU.mult)
                if ti % 4 == 0:
                    lnxts.append(lnxtp.tile([128, DT, 4, 128], BF16,
                                            name="lnxt", tag="lnxt"))
                nc.sync.dma_start_transpose(
                    out=lnxts[ti // 4][:, :, ti % 4, :], in_=px)

            for i in range(4 * N_CH):
                r0 = i * 128
                # bf16 x via SWDGE cast DMA: halves the 8MB input stream
                x_t = xin.tile([128, DIM], BF16, tag="x")
                nc.gpsimd.dma_start(out=x_t, in_=x_ext.ap()[r0:r0 + 128, :])
                if i == 3:
                    nc.gpsimd.dma_start(
                        out=w_sb[:, 0:DT // 2, :],
                        in_=wqkv_ext.ap().rearrange(
                            "(dt p) c -> p dt c", p=128)[:, 0:DT // 2, :])
                if i == 6:
                    nc.gpsimd.dma_start(
                        out=w_sb[:, DT // 2:DT, :],
                        in_=wqkv_ext.ap().rearrange(
                            "(dt p) c -> p dt c", p=128)[:, DT // 2:DT, :])
                st = statsp.tile([128, 2, 6], F32, tag="bn")
                for sg in range(2):
                    nc.vector.bn_stats(out=st[:, sg, :],
                                       in_=x_t[:, sg * 512:(sg + 1) * 512])
                mv = statsp.tile([128, 2], F32, tag="mv")
                nc.vector.bn_aggr(out=mv, in_=st)
                rstd = statsp.tile([128, 1], F32, tag="rstd")
                nc.scalar.activation(out=rstd, in_=mv[:, 1:2], func=AF.Sqrt,
                                     bias=eps_sb, scale=1.0)
                # recip+centering deferred one tile: the DVE queue runs the
                # next tile's bn_stats while ACT's sqrt is in flight, instead
                # of stalling on the cross-engine round trip
                if pend_center is not None:
                    center_flush()
                pend_center = (x_t, mv, rstd, i - 1)
                xts.append(x_t)
                if i == 3:
                    # tile 3 gates chunk 0's transposes (PE start): flush its
                    # centering immediately instead of deferring one tile
                    center_flush()
                    pend_center = None
            center_flush()
            # v_aug constant regions on Pool, WAW-gated behind x tile 7 so
            # the scheduler can't hoist them to the queue head where they
            # would delay the x input desc-gen (Pool) or bn_stats (DVE)
            nc.gpsimd.tensor_copy(v_aug[0:1, 0:1, 0:1, 0:1],
                                  xts[7][0:1, 0:1])
            nc.gpsimd.memset(v_aug[:, :, :, 0:64], 0.0)
            nc.gpsimd.memset(v_aug[:, :, :, 0:1], 1.0)

            # ---------------- phase 1 stage B: transpose + qkv ----------
            # the v path (qkv-v psum -> vt -> transpose -> v_aug) is deferred
            # one (ch,X) step so PE never waits on the drain round trip, and
            # all v drains/copies live on ACT: DVE's in-order queue is full
            # of bn_stats until ~35us and would hold the PSUM ring hostage.
            pend_vt = None

            def flush_vt():
                pX, pvt, pch = pend_vt
                for blk in range(4):
                    t = pch * 4 + blk
                    v_ps = ps1.tile([128, 128], BF16, tag="trsc")
                    nc.tensor.transpose(v_ps, pvt[:, blk * 128:(blk + 1) * 128],
                                        ident)
                    # later chunks' copies on DVE: its queue is clear of
                    # bn_stats by then, and ACT is phase-1 co-critical
                    cp = nc.scalar.copy if pch < 2 else nc.vector.tensor_copy
                    with nc.allow_low_precision(reason="bf16 v"):
                        cp(v_aug[:, 2 * pX:2 * pX + 2, t, 64:128],
                           v_ps.rearrange("p (h d) -> p h d", h=2))

            for ch in range(N_CH):
                lnxt = lnxts[ch]
                for db in range(DT):
                    # LN affine in place on the xbar-transposed tile (gamma,
                    # beta are per-partition here) on ACT
                    with nc.allow_low_precision(reason="bf16 activations"):
                        nc.scalar.activation(out=lnxt[:, db], in_=lnxt[:, db],
                                             func=AF.Identity,
                                             bias=beta_sb[:, db:db + 1],
                                             scale=gamma_sb[:, db:db + 1])

                # qkv matmuls: single-bank accumulation passes
                for X in range(2):
                    sl = slice(ch * RCHUNK, (ch + 1) * RCHUNK)
                    vt = vtp.tile([128, RCHUNK], BF16)
                    qkv_dst = [kT[X][:, sl], qT[X][:, sl], vt]
                    for j, jc in enumerate([1, 0, 2]):  # k first, then q, v
                        ct = jc * 2 + X
                        qkv_ps = ps1.tile([128, RCHUNK], F32, tag="trsc",
                                          name=f"qkv_ps_{ch}_{X}_{j}")
                        for db in range(DT):
                            nc.tensor.matmul(
                                qkv_ps,
                                w_sb[:, db, ct * 128:(ct + 1) * 128],
                                lnxt[:, db],
                                start=(db == 0), stop=(db == DT - 1))
                        with nc.allow_low_precision(reason="bf16 q/k"):
                            (nc.scalar.copy if ch < 2
                             else nc.vector.tensor_copy)(qkv_dst[j], qkv_ps)
                    if pend_vt is not None:
                        flush_vt()
                    pend_vt = (X, vt, ch)
            flush_vt()

            # ---------------- phase 2: attention (per head) ----------------
            a2a_in = [dram.tile([8, 64, 256], BF16, name=f"a2a_in{h}")
                      for h in range(H_LOC)]
            a2a_out = [dram.tile([8, 64, 256], BF16, name=f"a2a_out{h}")
                       for h in range(H_LOC)]

            # outT[P] = assembled lhsT for out-proj pair P: partitions
            # 64i:64i+64 <- head 2P+i, [p, q(srcquad), H(srcbatch), r]
            outT = [singles.tile([128, DT // 2, 2, 256], BF16, name=f"outT{i}")
                    for i in range(2)]

            def divide_pre(o_ps):
                # reciprocal of the fused denominator row + gpsimd broadcast
                r1 = divp.tile([1, 512], BF16, tag="r1", bufs=4)
                with nc.allow_low_precision(reason="bf16 denom"):
                    nc.vector.reciprocal(out=r1, in_=o_ps[0:1, :])
                rb = divp.tile([128, 512], BF16, tag="rb", bufs=4)
                nc.gpsimd.partition_broadcast(out_ap=rb, in_ap=r1)
                return rb

            def divide_mult(X, i, qc, o_ps, rb):
                # deferred one chunk so the in-order DVE queue never waits on
                # the slow gpsimd broadcast
                qsl = slice(qc * 512, (qc + 1) * 512)
                nc.vector.tensor_tensor(out=oh[X][64:128, i, qsl],
                                        in0=o_ps[64:128, :],
                                        in1=rb[64:128, :],
                                        op=ALU.mult)

            # exp engine round-robin per key tile: ACT(hw exp) / DVE / Pool
            # (Schraudolph on DVE+Pool). 6 ACT : 6 DVE : 4 Pool per chunk.
            # per-keytile score tiles (1 PSUM bank each, ring 6) give PE a
            # 6-tile stall horizon over the exp latency.
            EXP_PAT = ['A', 'D', 'A', 'D', 'A', 'D', 'A', 'A'] * 2

            pend_div = None
            for h in range(H_LOC):
                X, i = h // 2, h % 2
                psl = slice(64 * i, 64 * (i + 1))
                for qc in range(NQC):
                    qsl = slice(qc * 512, (qc + 1) * 512)
                    o_ps = ps_ov.tile([128, 512], F32,
                                      name=f"ov_{h}_{qc}", tag="ov")
                    for t in range(NKT):
                        ksl = slice(t * 128, (t + 1) * 128)
                        s_ps = ps1.tile([128, 512], F32,
                                        name=f"sc_{h}_{qc}_{t}", tag="trsc")
                        nc.tensor.matmul(s_ps,
                                         kT[X][psl, ksl], qT[X][psl, qsl],
                                         start=True, stop=True,
                                         tile_position=(64 * i, 0))
                        eng = EXP_PAT[t]
                        if eng == 'A':
                            ex = expsp.tile([128, 512], BF16)
                            nc.scalar.activation(out=ex, in_=s_ps,
                                                 func=AF.Exp,
                                                 bias=0.0, scale=SCALE)
                        else:
                            # one-op Schraudolph fast exp in bf16 bit-space
                            ex_i = expsp.tile([128, 512], I16, tag="exi")
                            nc.vector.tensor_scalar(
                                out=ex_i, in0=s_ps,
                                scalar1=A_SCH, scalar2=B_SCH,
                                op0=ALU.mult, op1=ALU.add)
                            ex = ex_i.bitcast(BF16)
                        nc.tensor.matmul(o_ps,
                                         v_aug[:, h, t, 0:128],
                                         ex,
                                         start=(t == 0), stop=(t == NKT - 1))
                    rb = divide_pre(o_ps)
                    if pend_div is not None:
                        divide_mult(*pend_div)
                    pend_div = (X, i, qc, o_ps, rb)

                # flush the deferred multiply before this head's A2A input
                # DMAs (they read oh)
                divide_mult(*pend_div)
                pend_div = None

                for j in range(8):
                    nc.sync.dma_start(
                        out=a2a_in[h][j, :, :],
                        in_=oh[X][64:128, i, j * 256:(j + 1) * 256])
                if h == 0:
                    # w_out bf16 cast-load on SWDGE. The WAW gate (1-element
                    # write into wo_sb that reads the last centered x tile)
                    # keeps the scheduler from hoisting this dep-free 4MB
                    # read to t=0, where it would starve the x input stream
                    # that gates LayerNorm; gated, it runs in Pool's idle
                    # window between the x stream and the attention exps.
                    nc.gpsimd.tensor_copy(wo_sb[0:1, 0:1, 0:1],
                                          xts[-1][0:1, 0:1])
                    nc.gpsimd.dma_start(
                        out=wo_sb,
                        in_=wout_ext.ap().rearrange("(it p) c -> p it c",
                                                    p=128))
                    nc.sync.dma_start(out=bo_f32,
                                      in_=bout_ext.ap().unsqueeze(0))
                    nc.vector.tensor_copy(bo_sb, bo_f32)
                nc.gpsimd.collective_compute(
                    "AllToAll", ALU.bypass,
                    replica_groups=[[0, 1, 2, 3, 4, 5, 6, 7]],
                    ins=[a2a_in[h].opt()], outs=[a2a_out[h].opt()])
                # assemble this head's half of outT right after its
                # collective. All four go on HWDGE: pass A's wait threshold
                # only counts the queue up to outT[0]'s assembly (h==1),
                # which precedes the h==3 DMAs, so pass A is not gated on
                # the last collective.
                for H in range(2):
                    if h == 3:
                        # split the last assembly per 128-row half: pass-B
                        # groups release progressively instead of as one
                        # cold batch after the final collective
                        for rh in range(2):
                            nc.sync.dma_start(
                                out=outT[X][psl, :, H, rh * 128:(rh + 1) * 128],
                                in_=a2a_out[h][4 * H:4 * (H + 1), :,
                                               rh * 128:(rh + 1) * 128
                                               ].rearrange("q p r -> p q r"))
                    else:
                        nc.sync.dma_start(
                            out=outT[X][psl, :, H, :],
                            in_=a2a_out[h][4 * H:4 * (H + 1)].rearrange(
                                "q p r -> p q r"))

            # ---------------- phase 3: out proj ----------
            # pass A: pair-0 inner tiles + bias as CLOSED accumulation groups
            # (overlaps the in-flight last A2A), partials drained to SBUF;
            # pass B: pair-1 tiles after A2A #3, combine add replaces drain.
            ps_ov_cm.__exit__(None, None, None); ps1_cm.__exit__(None, None, None)
            ps_op_cm = tc.tile_pool(name="ps_op", bufs=4, space="PSUM")
            ps_op = ps_op_cm.__enter__()
            e_parts = {}
            for rt in range(4):
                for oc in range(2):
                    ep = ps_op.tile([128, 512], F32, tag="op",
                                    name=f"ep_{rt}_{oc}")
                    for q in range(DT // 2):
                        nc.tensor.matmul(
                            ep,
                            outT[0][:, q, rt // 2,
                                    (rt % 2) * 128:(rt % 2) * 128 + 128],
                            wo_sb[:, q * 2, oc * 512:(oc + 1) * 512],
                            start=(q == 0), stop=False)
                    nc.tensor.matmul(
                        ep, ones_bf, bo_sb[:, oc * 512:(oc + 1) * 512],
                        start=False, stop=True)
                    e_sb = osbp.tile([128, 512], BF16, tag="e_sb", bufs=8,
                                     name=f"e_sb_{rt}_{oc}")
                    # alternate drains DVE/ACT to halve the drain tail
                    if (rt + oc) % 2 == 0:
                        nc.vector.tensor_copy(e_sb, ep)
                    else:
                        with nc.allow_low_precision(reason="bf16 partials"):
                            nc.scalar.copy(e_sb, ep)
                    e_parts[(rt, oc)] = e_sb
            for rt in range(4):
                for oc in range(2):
                    op_ps = ps_op.tile([128, 512], F32, tag="op",
                                       name=f"op_ps_{rt}_{oc}")
                    # 256-col halves: the cold-p-state pricing after the
                    # collective wait covers a fixed NUMBER of queued
                    # instructions, so halving each matmul halves the time
                    # spent at the throttled clock
                    colw = 32
                    for colh in range(512 // colw):
                        csl = slice(oc * 512 + colh * colw,
                                    oc * 512 + (colh + 1) * colw)
                        for q in range(DT // 2):
                            nc.tensor.matmul(
                                op_ps[:, colh * colw:(colh + 1) * colw],
                                outT[1][:, q, rt // 2,
                                        (rt % 2) * 128:(rt % 2) * 128 + 128],
                                wo_sb[:, q * 2 + 1, csl],
                                start=(q == 0), stop=(q == DT // 2 - 1))
                    o_sb = osbp.tile([128, 512], BF16, tag="o_sb", bufs=4)
                    with nc.allow_low_precision(reason="bf16 output"):
                        nc.vector.tensor_tensor(out=o_sb, in0=op_ps,
                                                in1=e_parts[(rt, oc)],
                                                op=ALU.add)
                    nc.sync.dma_start(
                        out=out_ext.ap()[rt * 128:(rt + 1) * 128,
                                         oc * 512:(oc + 1) * 512], in_=o_sb)
            ps_op_cm.__exit__(None, None, None)

    nc.compile()
    return nc


def _make_in_maps(inputs):
    x = np.ascontiguousarray(
        np.asarray(inputs["x"], dtype=np.float32).reshape(B * N, DIM))
    gamma = np.asarray(inputs["gamma"], dtype=np.float32)
    beta = np.asarray(inputs["beta"], dtype=np.float32)
    w_qkv = np.asarray(inputs["w_qkv"], dtype=np.float32)
    w_out = np.ascontiguousarray(np.asarray(inputs["w_out"], dtype=np.float32))
    b_out = np.asarray(inputs["b_out"], dtype=np.float32)

    in_maps = []
    for c in range(N_CORES):
        b = c // 4
        qd = c % 4
        cols = []
        for j in range(3):
            cols.append(w_qkv[:, j * DIM + qd * 256:(j * DIM + qd * 256) + 256])
        wqkv_s = np.ascontiguousarray(np.concatenate(cols, axis=1))
        in_maps.append(dict(
            x=np.ascontiguousarray(x[b * N:(b + 1) * N]),
            gamma=gamma, beta=beta,
            wqkv=wqkv_s, wout=w_out, bout=b_out))
    return in_maps


def kernel(x, gamma, beta, w_qkv, w_out, b_out):
    global _CACHED_NC
    if _CACHED_NC is None:
        _CACHED_NC = build()
    nc = _CACHED_NC
    in_maps = _make_in_maps(dict(x=x, gamma=gamma, beta=beta, w_qkv=w_qkv,
                                 w_out=w_out, b_out=b_out))
    res = run_bass_kernel_spmd(nc, in_maps, core_ids=list(range(N_CORES)))
    # core c's "out" [512, 1024] = rows [256c, 256c+256) of batch 0 then batch 1
    out = np.empty((B, N, DIM), dtype=np.float32)
    for c in range(N_CORES):
        o = np.asarray(res.results[c]["out"], dtype=np.float32)
        out[0, 256 * c:256 * (c + 1)] = o[0:256]
        out[1, 256 * c:256 * (c + 1)] = o[256:512]
    return out
